# revision 6
# baseline (speedup 1.0000x reference)
"""CPN forward on 8 Trainium2 cores — f32r screen + bit-exact PE rescore.

Two-tier winner search, bit-exact vs the reference-on-neuron:
  1. SCREEN (fast): full [128,4096] score tile via float32r matmuls (full PE
     rate, tf32-class error ~3e-4 abs) minus w2; DVE max/max_index yields the
     top-8 candidate indices per row. The true winner is within the top R=4
     screened candidates unless >=4 rivals sit within ~1e-3 of the winner
     (probability ~1e-12 per row for this data).
  2. RESCORE (exact): gather the R candidate codeword rows (+their w2) from
     DRAM, PE-transpose them into K-major columns, and redo the dot products
     on the PE in fp32 with the same K-tile order as a full matmul — the PE
     accumulation chain is then bit-identical to the reference's matmul.
     Apply the reference's exact fp32 rounding chain on the R candidate
     scores:  t1=fl(x2+w2); sq=fl(t1-2dot); d=ACT-Sqrt(sq)  (bit-identical
     to jnp-on-neuron sqrt). Winner = min index among ranks tied at min d —
     exactly jnp.argmin's first-index-tie rule.
Output gather: indirect-DMA rows of grossberg.T by winner index.
"""
import numpy as np

import concourse.bacc as bacc
import concourse.bass as bass
import concourse.mybir as mybir
import concourse.tile as tile
from concourse.bass_utils import run_bass_kernel_spmd
from concourse.masks import make_identity

F32 = mybir.dt.float32
F32R = mybir.dt.float32r
I32 = mybir.dt.int32
U32 = mybir.dt.uint32

B = 8192
D = 512
H = 4096
O = 1024
NCORES = 8
BS = B // NCORES   # 1024 rows per core
BT = 128
NBT = BS // BT     # 8
NT = 512
NNT = H // NT      # 8
KT = D // 128      # 4
R = 3              # rescored candidates per row
KA = 520           # kaug row: 512 weights + w2 + 7 pad

_CACHED_NC = None


def _build_nc():
    nc = bacc.Bacc("TRN2", target_bir_lowering=False, debug=False)

    xT_d = nc.declare_dram_parameter("xT", [D, BS], F32, False)   # 2*x_shard^T
    kT_d = nc.declare_dram_parameter("kT", [D, H], F32, False)
    w2n_d = nc.declare_dram_parameter("w2n", [1, H], F32, False)  # -w2 row
    x2_d = nc.declare_dram_parameter("x2", [BT, NBT], F32, False)
    kaug_d = nc.declare_dram_parameter("kaug", [H, KA], F32, False)
    gT_d = nc.declare_dram_parameter("gT", [H, O], F32, False)
    out_d = nc.declare_dram_parameter("out", [BS, O], F32, True)
    idx_d = nc.declare_dram_parameter("idx", [BS], I32, True)

    with tile.TileContext(nc) as tc:
        with (
            tc.tile_pool(name="wpool", bufs=1) as wpool,
            tc.tile_pool(name="mpool", bufs=2) as mpool,
            tc.tile_pool(name="cpool", bufs=2) as cpool,
            tc.tile_pool(name="ctpool", bufs=2) as ctpool,
            tc.tile_pool(name="spool", bufs=8) as spool,
            tc.tile_pool(name="scr8", bufs=4) as scr8,
            tc.tile_pool(name="gpool", bufs=3) as gpool,
            tc.tile_pool(name="pspool", bufs=4, space="PSUM") as pspool,
            tc.tile_pool(name="tppool", bufs=2, space="PSUM") as tppool,
            tc.tile_pool(name="prpool", bufs=2, space="PSUM") as prpool,
        ):
            # --- constants / weights ---
            kT_r = wpool.tile([128, KT * H], F32R)
            xT_sb = wpool.tile([128, KT * BS], F32)
            xT_r = wpool.tile([128, KT * BS], F32R)
            w2n_sb = wpool.tile([1, H], F32)
            w2n_r = wpool.tile([1, H], F32R)
            ones_sb = wpool.tile([1, 128], F32)
            ones_r = wpool.tile([1, 128], F32R)
            x2_sb = wpool.tile([128, NBT], F32)
            ident = wpool.tile([128, 128], F32)
            make_identity(nc, ident[:])
            nc.vector.memset(ones_sb[:], 1.0)
            nc.scalar.copy(ones_r[:], ones_sb[:])
            # stage the f32->f32r rounding of kT through the (not yet used)
            # score-tile slots to avoid a dedicated staging pool
            for k in range(KT):
                ktmp = mpool.tile([128, H], F32, name=f"ktmp_{k}", tag="m")
                nc.sync.dma_start(out=ktmp[:], in_=kT_d[k * 128:(k + 1) * 128, :])
                nc.scalar.copy(kT_r[:, k * H:(k + 1) * H], ktmp[:])
                nc.sync.dma_start(
                    out=xT_sb[:, k * BS:(k + 1) * BS],
                    in_=xT_d[k * 128:(k + 1) * 128, :],
                )
            nc.scalar.copy(xT_r[:], xT_sb[:])
            nc.sync.dma_start(out=w2n_sb[:], in_=w2n_d[:])
            nc.scalar.copy(w2n_r[:], w2n_sb[:])
            nc.sync.dma_start(out=x2_sb[:], in_=x2_d[:])

            for bt in range(NBT):
                # ---- screen: full f32r scores, top-8 per row ----
                m_scr = mpool.tile([128, H], F32, name=f"m_{bt}", tag="m")
                for nt in range(NNT):
                    ps = pspool.tile([128, NT], F32, name=f"ps_{bt}_{nt}", tag="ps")
                    for k in range(KT):
                        nc.tensor.matmul(
                            ps[:],
                            lhsT=xT_r[:, k * BS + bt * BT: k * BS + (bt + 1) * BT],
                            rhs=kT_r[:, k * H + nt * NT: k * H + (nt + 1) * NT],
                            start=(k == 0),
                            stop=False,
                        )
                    # rank-1 bias row: scores -= w2 (K=1 f32r matmul)
                    nc.tensor.matmul(
                        ps[:],
                        lhsT=ones_r[0:1, :],
                        rhs=w2n_r[0:1, nt * NT:(nt + 1) * NT],
                        start=False,
                        stop=True,
                    )
                    nc.scalar.copy(m_scr[:, nt * NT:(nt + 1) * NT], ps[:])
                mx8 = scr8.tile([128, 8], F32, name=f"mx8_{bt}", tag="mx8")
                ix8 = scr8.tile([128, 8], U32, name=f"ix8_{bt}", tag="ix8")
                nc.vector.max(mx8[:], m_scr[:])
                nc.vector.max_index(ix8[:], mx8[:], m_scr[:])

                # ---- rescore candidates exactly on the PE ----
                cand = cpool.tile([128, R * KA], F32, name=f"cand_{bt}", tag="cand")
                ixr = scr8.tile([128, R], I32, name=f"ixr_{bt}", tag="ixr")
                nc.vector.tensor_copy(ixr[:], ix8[:, 0:R])
                for r in range(R):
                    nc.gpsimd.indirect_dma_start(
                        out=cand[:, r * KA:(r + 1) * KA],
                        out_offset=None,
                        in_=kaug_d[:],
                        in_offset=bass.IndirectOffsetOnAxis(
                            ap=ixr[:, r:r + 1], axis=0
                        ),
                    )
                # transpose candidate rows into K-major columns
                candT = ctpool.tile(
                    [128, KT * R * 128], F32, name=f"candT_{bt}", tag="candT"
                )
                for kk in range(KT):
                    tpb = tppool.tile(
                        [128, R * 128], F32, name=f"tpb_{bt}_{kk}", tag="tpb"
                    )
                    for r in range(R):
                        nc.tensor.transpose(
                            tpb[:, r * 128:(r + 1) * 128],
                            cand[:, r * KA + kk * 128: r * KA + (kk + 1) * 128],
                            ident[:],
                        )
                    nc.scalar.copy(
                        candT[:, kk * R * 128:(kk + 1) * R * 128], tpb[:]
                    )
                # fp32 rescore matmul — bit-identical accumulation chain
                pr = prpool.tile([128, R * 128], F32, name=f"pr_{bt}", tag="pr")
                for kk in range(KT):
                    nc.tensor.matmul(
                        pr[:],
                        lhsT=xT_sb[:, kk * BS + bt * BT: kk * BS + (bt + 1) * BT],
                        rhs=candT[:, kk * R * 128:(kk + 1) * R * 128],
                        start=(kk == 0),
                        stop=(kk == KT - 1),
                    )
                # extract per-rank diagonal: pdiag[b, r] = pr[b, r*128+b]
                pdiag = scr8.tile([128, R], F32, name=f"pdiag_{bt}", tag="pdiag")
                for r in range(R):
                    ttr_out = spool.tile(
                        [128, 128], F32, name=f"ttro_{bt}_{r}", tag="ttro"
                    )
                    nc.vector.tensor_mul(
                        ttr_out[:], pr[:, r * 128:(r + 1) * 128], ident[:]
                    )
                    nc.vector.tensor_reduce(
                        pdiag[:, r:r + 1], ttr_out[:],
                        axis=mybir.AxisListType.X, op=mybir.AluOpType.add,
                    )
                # reference's exact fp32 chain on the R candidate scores
                w2c = scr8.tile([128, R], F32, name=f"w2c_{bt}", tag="w2c")
                for r in range(R):
                    nc.vector.tensor_copy(
                        w2c[:, r:r + 1], cand[:, r * KA + 512: r * KA + 513]
                    )
                t1c = scr8.tile([128, R], F32, name=f"t1c_{bt}", tag="t1c")
                nc.scalar.add(t1c[:], w2c[:], x2_sb[:, bt:bt + 1])
                s2c = scr8.tile([128, R], F32, name=f"s2c_{bt}", tag="s2c")
                nc.vector.tensor_tensor(
                    out=s2c[:], in0=t1c[:], in1=pdiag[:],
                    op=mybir.AluOpType.subtract,
                )
                dc = scr8.tile([128, R], F32, name=f"dc_{bt}", tag="dc")
                nc.scalar.sqrt(dc[:], s2c[:])
                # winner = min candidate index among ranks tied at min d
                dmin = scr8.tile([128, 1], F32, name=f"dmin_{bt}", tag="dmin")
                nc.vector.tensor_reduce(
                    dmin[:], dc[:], axis=mybir.AxisListType.X,
                    op=mybir.AluOpType.min,
                )
                eqm = scr8.tile([128, R], F32, name=f"eqm_{bt}", tag="eqm")
                nc.vector.tensor_tensor(
                    out=eqm[:], in0=dc[:],
                    in1=dmin[:, 0:1].to_broadcast([128, R]),
                    op=mybir.AluOpType.is_equal,
                )
                ixf = scr8.tile([128, R], F32, name=f"ixf_{bt}", tag="ixf")
                nc.vector.tensor_copy(ixf[:], ix8[:, 0:R])
                pen = scr8.tile([128, R], F32, name=f"pen_{bt}", tag="pen")
                nc.vector.tensor_scalar_mul(pen[:], eqm[:], -1.0e7)
                nc.vector.tensor_scalar_add(pen[:], pen[:], 1.0e7)
                hsel = scr8.tile([128, R], F32, name=f"hsel_{bt}", tag="hsel")
                nc.vector.tensor_add(hsel[:], pen[:], ixf[:])
                hval = scr8.tile([128, 1], F32, name=f"hval_{bt}", tag="hval")
                nc.vector.tensor_reduce(
                    hval[:], hsel[:], axis=mybir.AxisListType.X,
                    op=mybir.AluOpType.min,
                )
                hwin = scr8.tile([128, 1], I32, name=f"hwin_{bt}", tag="hwin")
                nc.vector.tensor_copy(hwin[:], hval[:])

                # ---- output gather + stores ----
                g_sb = gpool.tile([128, O], F32, name=f"g_{bt}", tag="g")
                nc.gpsimd.indirect_dma_start(
                    out=g_sb[:],
                    out_offset=None,
                    in_=gT_d[:],
                    in_offset=bass.IndirectOffsetOnAxis(ap=hwin[:, :1], axis=0),
                )
                nc.sync.dma_start(
                    out=out_d[bt * BT:(bt + 1) * BT, :], in_=g_sb[:]
                )
                nc.sync.dma_start(out=idx_d[bt * BT:(bt + 1) * BT], in_=hwin[:, 0])

    nc.compile()
    return nc


def get_nc():
    global _CACHED_NC
    if _CACHED_NC is None:
        _CACHED_NC = _build_nc()
    return _CACHED_NC


def _row_norms_sq(a):
    """fp32 row norms matching jnp.sum(a*a, axis=1) on this backend."""
    try:
        import jax.numpy as jnp

        return np.asarray(jnp.sum(jnp.asarray(a) * jnp.asarray(a), axis=1))
    except Exception:
        return np.sum(a.astype(np.float32) ** 2, axis=1, dtype=np.float32)


def make_in_maps(x, kohonen_weights, grossberg_weights):
    kT = np.ascontiguousarray(kohonen_weights.T)                  # [D, H]
    w2 = _row_norms_sq(kohonen_weights).astype(np.float32)        # [H]
    x2 = _row_norms_sq(x).astype(np.float32)                      # [B]
    w2n = np.ascontiguousarray(-w2.reshape(1, H))                 # [1, H]
    kaug = np.zeros((H, KA), dtype=np.float32)
    kaug[:, :512] = kohonen_weights
    kaug[:, 512] = w2
    gT = np.ascontiguousarray(grossberg_weights.T)                # [H, O]
    in_maps = []
    for c in range(NCORES):
        xs = x[c * BS:(c + 1) * BS]
        x2s = x2[c * BS:(c + 1) * BS]
        in_maps.append({
            "xT": np.ascontiguousarray(2.0 * xs.T),
            "kT": kT,
            "w2n": w2n,
            "x2": np.ascontiguousarray(x2s.reshape(NBT, BT).T),
            "kaug": kaug,
            "gT": gT,
        })
    return in_maps


def kernel(x, kohonen_weights, grossberg_weights):
    x = np.asarray(x, dtype=np.float32)
    kohonen_weights = np.asarray(kohonen_weights, dtype=np.float32)
    grossberg_weights = np.asarray(grossberg_weights, dtype=np.float32)

    nc = get_nc()
    in_maps = make_in_maps(x, kohonen_weights, grossberg_weights)
    res = run_bass_kernel_spmd(nc, in_maps, list(range(NCORES))).results

    output = np.concatenate([res[c]["out"] for c in range(NCORES)], axis=0)
    winners = np.concatenate([res[c]["idx"] for c in range(NCORES)], axis=0)
    return output, winners.astype(np.int32)
